# revision 1
# baseline (speedup 1.0000x reference)
"""EntropyLinear Trainium2 kernel (8-core SPMD, batch-sharded).

Computes, for x[B,IN], weight[C,OUT,IN], bias[C,1,OUT]:
    gamma[c,i]      = sum_o |W[c,o,i]|
    alpha_norm[c,i] = exp((gamma[c,i] - max_i gamma[c,i]) / T)
                      (softmax / rowmax(softmax) -- denominator cancels)
    y[b,c,o]        = sum_i x[b,i] * alpha_norm[c,i] * W[c,o,i] + bias[c,0,o]

Strategy: data-parallel over batch. Each of the 8 cores gets 1024 rows of x
plus the full weight/bias (tiny), folds alpha_norm into a transposed copy of
W on-chip, and runs the per-class GEMM as one [1024,256] x [256,1600] f32r
matmul.

The whole prologue is pipelined per W piece (4 triples of 128-row groups +
the 64-row tail). Gamma rows for a class depend only on that class's own
128 W rows, so each piece independently runs:
  DMA -> ACT abs -> PE block-mask matmul (partition-reduce over o, into a
  per-class-row PSUM accumulator) -> DVE rowmax -> ACT exp -> PE transpose
  of its alpha rows -> DVE scale-fused W-transpose copies (wm) -> the
  matching output chunk of b-tile 0 -> its half of the first store.
Stores then stream back-to-back; steady-state epilogue alternates DVE
fused psum+bias adds (2 of 5 chunks) and ACT plain copies whose bias is
pre-injected in PSUM by a rank-1 (ones x bias) matmul. A burst of junk
transposes warms the PE clock gate during the load window.
"""

import os
from contextlib import ExitStack

import numpy as np

import concourse.bass as bass
import concourse.tile as tile
from concourse import masks, mybir
from concourse.bass_utils import run_bass_kernel_spmd

# ---------------------------------------------------------------------------
# Workaround for this walrus build's 1-sync-wait-per-instruction limit:
# Tile's scheduler (and its exit drain) may attach several semaphore waits to
# one instruction; walrus codegen rejects that ("Too many sync wait
# commands"). Post-pass: hoist extra waits onto same-engine NOPs emitted
# immediately before the instruction (same engine stream => same semantics).
# ---------------------------------------------------------------------------


def _split_multi_waits(nc):
    for fn in nc.m.functions:
        for blk in fn.blocks:
            out = []
            changed = False
            for inst in list(blk.instructions):
                si = inst.sync_info
                waits = list(si.on_wait) if si is not None and si.on_wait else []
                if len(waits) > 1:
                    changed = True
                    for w in waits[:-1]:
                        nop = mybir.InstNoOp(
                            name=nc.get_next_instruction_name(), ins=[], outs=[])
                        nop.engine = inst.engine
                        nop.sync_info = mybir.SyncInfo(on_wait=[w], on_update=[])
                        nc.register_instruction(nop)
                        out.append(nop)
                    upd = list(si.on_update) if si.on_update else []
                    inst.sync_info = mybir.SyncInfo(
                        on_wait=[waits[-1]], on_update=upd)
                out.append(inst)
            if changed:
                blk.instructions = out

# ---------------------------------------------------------------------------

B, IN, OUT, C = 8192, 256, 32, 50
TEMPERATURE = 0.6
N_CORES = 8
BS = B // N_CORES          # rows of x per core
CO = C * OUT               # 1600 fused (class, out) columns
F32 = mybir.dt.float32
F32R = mybir.dt.float32r
BF16 = mybir.dt.bfloat16

N_BT = BS // 128           # b-tiles per core
N_KH = IN // 128           # contraction chunks
CO_CHUNK = 320             # psum chunk (10 classes, 1280B -> one psum bank)
N_CC = CO // CO_CHUNK
N_GRP = (CO + 127) // 128  # W co-groups (13: 12 full + 1 half)
CPG = 128 // OUT           # classes per full W group (4)
GPP = int(os.environ.get("EL_GPP", "3"))  # W groups per pipeline piece
N_PC = (12 + GPP - 1) // GPP + 1           # full pieces + the 64-row tail

# "f32r" (full-rate fp32 path), "f32" (4x slower, exact), "bf16"
MM_MODE = os.environ.get("EL_MM_MODE", "f32r")
Y_DVE_OF_5 = int(os.environ.get("EL_Y_DVE_OF_5", "2"))   # y chunks on DVE per 5
XCOPY_ACT = os.environ.get("EL_XCOPY_ACT", "0") == "1"   # x copies on ACT
T0_ACT = os.environ.get("EL_T0_ACT", "1") == "1"         # t=0 all-ACT epilogue

_CACHE = {}


def _build(mode, repeat=1):
    nc = bass.Bass(trn_type="TRN2", target_bir_lowering=False, debug=False,
                   num_devices=N_CORES)
    x_d = nc.dram_tensor("x", [BS, IN], F32, kind="ExternalInput").ap()
    w_d = nc.dram_tensor("weight", [C, OUT, IN], F32, kind="ExternalInput").ap()
    b_d = nc.dram_tensor("bias", [C, 1, OUT], F32, kind="ExternalInput").ap()
    y_d = nc.dram_tensor("y", [BS, C, OUT], F32, kind="ExternalOutput").ap()

    w_flat = w_d.rearrange("c o i -> (c o) i")      # [1600, 256]
    y_flat = y_d.rearrange("b c o -> b (c o)")      # [BS, 1600]

    # dtype of matmul operand tiles; f32r rounding happens in the DVE ops
    # that produce them
    mm_dt = {"f32": F32, "f32r": F32R, "bf16": BF16}[mode]

    with tile.TileContext(nc) as tc, ExitStack() as ctx:
      const_p = ctx.enter_context(tc.tile_pool(name="const", bufs=1))
      wn_p = ctx.enter_context(tc.tile_pool(name="wn", bufs=1))
      small_p = ctx.enter_context(tc.tile_pool(name="small", bufs=1))
      x_p = ctx.enter_context(tc.tile_pool(name="x", bufs=1))
      xt_p = ctx.enter_context(tc.tile_pool(name="xt", bufs=16))
      y_p = ctx.enter_context(tc.tile_pool(name="y", bufs=3))
      ps_tp = ctx.enter_context(tc.tile_pool(name="ps_tp", bufs=2, space="PSUM"))
      ps_g = ctx.enter_context(tc.tile_pool(name="ps_g", bufs=2, space="PSUM"))
      ps_y = ctx.enter_context(tc.tile_pool(name="ps_y", bufs=4, space="PSUM"))
      rep_cm = tc.For_i(0, repeat, 1) if repeat > 1 else None
      if rep_cm is not None:
          rep_cm.__enter__()
      if True:
          # ---- constants ----
          ident = const_p.tile([128, 128], F32, tag="ident")
          masks.make_identity(nc, ident[:])
          # Sliding-window mask for the per-class partition reduction.
          # mstore[p, CW + j] = 1 iff p // OUT == j; all else 0. Group g uses
          # the [128, CW] window starting at column CW - CPG*g, which places
          # the identity block exactly at out-classes CPG*g..CPG*g+CPG.
          CW = 32     # per-piece gamma psum partitions (>= GPP*CPG)
          mstore_f = const_p.tile([128, 2 * CW], F32, tag="mstore_f")
          nc.vector.memset(mstore_f[:], 0.0)
          # block-identity derived by summing OUT-wide column blocks of the
          # identity matrix
          nc.vector.tensor_reduce(
              mstore_f[:, CW:CW + CPG],
              ident[:].rearrange("p (j q) -> p j q", q=OUT),
              axis=mybir.AxisListType.X, op=mybir.AluOpType.add)
          mstore = const_p.tile([128, 2 * CW], F32R, tag="mstore")
          nc.vector.tensor_copy(mstore[:], mstore_f[:])
          ones_f = const_p.tile([1, 128], F32, tag="ones_f")
          nc.vector.memset(ones_f[:], 1.0)
          ones_r = const_p.tile([1, 128], F32R, tag="ones_r")
          nc.vector.tensor_copy(ones_r[:], ones_f[:])

          # ---- PE warmup: junk transposes so the HAM clock gate opens ----
          for _ in range(14):
              warm_ps = ps_tp.tile([128, 256], F32, tag="tp", name="tp")
              nc.tensor.transpose(warm_ps[:, :128], ident[:], ident[:])

          # ---- loads: W piece 0, x(t=0,1), W pieces 1-3 + tail, bias,
          #      x rest. Each W triple is one 3D-AP DMA. ----
          wbig = wn_p.tile([128, N_GRP * IN], F32, tag="wbig")
          xbig = x_p.tile([128, N_BT * IN], F32, tag="xbig")

          def load_w_piece(p):
              g0 = p * GPP
              g1 = min(g0 + GPP, 12)
              if g0 < 12:
                  nc.sync.dma_start(
                      wbig[:, g0 * IN:g1 * IN].rearrange(
                          "p (g i) -> p g i", i=IN),
                      w_flat[g0 * 128:g1 * 128, :].rearrange(
                          "(g p) i -> p g i", p=128))
              if g1 * 128 < CO <= (g0 + GPP) * 128:
                  nc.sync.dma_start(wbig[:64, 12 * IN:], w_flat[1536:1600, :])

          load_w_piece(0)
          nc.sync.dma_start(
              xbig[:, :2 * IN].rearrange("p (t i) -> p t i", i=IN),
              x_d[0:256, :].rearrange("(t p) i -> p t i", p=128))
          for p in range(1, N_PC):
              load_w_piece(p)
          bias_row = const_p.tile([1, CO], F32, tag="bias_row")
          nc.sync.dma_start(bias_row[:], b_d.rearrange("c u o -> u (c o)"))
          nc.sync.dma_start(
              xbig[:, 2 * IN:].rearrange("p (t i) -> p t i", i=IN),
              x_d[256:, :].rearrange("(t p) i -> p t i", p=128))

          wn = [wbig[:, g * IN:(g + 1) * IN] for g in range(N_GRP)]
          xn = [xbig[:, t * IN:(t + 1) * IN] for t in range(N_BT)]

          # ---- per-piece pipeline state ----
          awbig = wn_p.tile([128, N_GRP * IN], F32R, tag="awbig")
          ant = [small_p.tile([128, C], F32, tag=f"ant{h}", name=f"ant{h}")
                 for h in range(N_KH)]
          wm = [const_p.tile([128, CO], mm_dt, tag=f"wm{h}", name=f"wm{h}")
                for h in range(N_KH)]
          bias_r = const_p.tile([1, CO], F32R, tag="bias_r")
          nc.vector.tensor_copy(bias_r[:], bias_row[:])
          bias_rep = const_p.tile([128, CO], F32, tag="bias_rep")
          if not T0_ACT:
              for n in range(N_CC):
                  sl = slice(n * CO_CHUNK, (n + 1) * CO_CHUNK)
                  ps = ps_y.tile([128, CO_CHUNK], F32, tag="ps", name="ps")
                  nc.tensor.matmul(ps[:], ones_r[:], bias_r[:, sl],
                                   start=True, stop=True)
                  nc.scalar.copy(bias_rep[:, sl], ps[:])

          def x_transpose(t):
              xtt = []
              for h in range(N_KH):
                  tp = ps_tp.tile([128, 256], F32, tag="tp", name="tp")
                  nc.tensor.transpose(tp[:, :128],
                                      xn[t][:, h * 128:(h + 1) * 128],
                                      ident[:])
                  xte = xt_p.tile([128, 128], mm_dt, tag="xte", name="xte")
                  if XCOPY_ACT:
                      nc.scalar.copy(xte[:], tp[:, :128])
                  else:
                      nc.vector.tensor_copy(xte[:], tp[:, :128])
                  xtt.append(xte)
              return xtt

          def w_piece_pipeline(p):
              """abs -> gamma rows -> alpha rows -> anT cols -> wm cols."""
              g0 = p * GPP
              g1 = min(g0 + GPP, N_GRP)
              col0, col1 = g0 * 128, min(CO, g1 * 128)
              ncls = (col1 - col0) // OUT
              cl0 = g0 * CPG
              pgl = min(128, CO - (g1 - 1) * 128)      # rows in last group
              rows = slice(cl0, cl0 + ncls)
              # |W| for the piece (one wide ACT op; rows beyond pgl unused)
              nc.scalar.activation(awbig[:pgl, g0 * IN:g1 * IN],
                                   wbig[:pgl, g0 * IN:g1 * IN],
                                   mybir.ActivationFunctionType.Abs)
              # per-group partition reduction over o into this piece's
              # local gamma rows (base partition 0)
              gps = ps_g.tile([CW, IN], F32, tag="gps", name="gps")
              for g in range(g0, g1):
                  pg = min(128, CO - g * 128)
                  loc = g - g0
                  nc.tensor.matmul(
                      gps[:, :],
                      mstore[:pg, CW - CPG * loc: CW * 2 - CPG * loc],
                      awbig[:pg, g * IN:(g + 1) * IN],
                      start=(loc == 0), stop=(g == g1 - 1),
                      skip_group_check=True)
              # alpha rows for this piece's classes
              gm = small_p.tile([CW, 1], F32, tag="gm", name="gm", bufs=2)
              nc.vector.tensor_reduce(gm[:ncls], gps[:ncls, :],
                                      axis=mybir.AxisListType.X,
                                      op=mybir.AluOpType.max)
              nb = small_p.tile([CW, 1], F32, tag="nb", name="nb", bufs=2)
              nc.vector.tensor_scalar_mul(nb[:ncls], gm[:ncls],
                                          -1.0 / TEMPERATURE)
              anp = small_p.tile([CW, IN], F32, tag="anp", name="anp",
                                 bufs=2)
              nc.scalar.activation(anp[:ncls, :], gps[:ncls, :],
                                   mybir.ActivationFunctionType.Exp,
                                   bias=nb[:ncls], scale=1.0 / TEMPERATURE)
              # transpose alpha rows into anT columns, then scale-fused
              # W-transpose into wm
              for h in range(N_KH):
                  tp = ps_tp.tile([128, 256], F32, tag="tp", name="tp")
                  nc.tensor.transpose(tp[:, :ncls],
                                      anp[:ncls, h * 128:(h + 1) * 128],
                                      ident[:ncls, :ncls])
                  nc.vector.tensor_copy(ant[h][:, rows], tp[:, :ncls])
                  for ga in range(g0, g1, 2):
                      gb = min(ga + 2, g1)
                      bcol0, bcol1 = ga * 128, min(CO, gb * 128)
                      bncls = (bcol1 - bcol0) // OUT
                      bc0 = ga * CPG
                      tpw = ps_tp.tile([128, 256], F32, tag="tp", name="tp")
                      for g in range(ga, gb):
                          pg = min(128, CO - g * 128)
                          off = (g - ga) * 128
                          nc.tensor.transpose(
                              tpw[:, off:off + pg],
                              wn[g][:pg, h * 128:(h + 1) * 128],
                              ident[:pg, :pg])
                      nc.vector.tensor_tensor(
                          wm[h][:, bcol0:bcol1].rearrange(
                              "p (c o) -> p c o", o=OUT),
                          tpw[:, :bcol1 - bcol0].rearrange(
                              "p (c o) -> p c o", o=OUT),
                          ant[h][:, bc0:bc0 + bncls].unsqueeze(2).broadcast_to(
                              [128, bncls, OUT]),
                          op=mybir.AluOpType.mult)

          def y_chunk(t, n, y_sb, on_act):
              sl = slice(n * CO_CHUNK, (n + 1) * CO_CHUNK)
              ps = ps_y.tile([128, CO_CHUNK], F32, tag="ps", name="ps")
              if on_act:
                  nc.tensor.matmul(ps[:], ones_r[:], bias_r[:, sl],
                                   start=True, stop=False)
              for h in range(N_KH):
                  nc.tensor.matmul(ps[:], xt[t][h][:], wm[h][:, sl],
                                   start=(h == 0 and not on_act),
                                   stop=(h == N_KH - 1))
              if on_act:
                  nc.scalar.copy(y_sb[:, sl], ps[:])
              else:
                  nc.vector.tensor_tensor(y_sb[:, sl], ps[:],
                                          bias_rep[:, sl],
                                          op=mybir.AluOpType.add)

          # ---- pipelined prologue: piece p unlocks t=0's chunk p ----
          # chunk n spans co columns [320n, 320n+320) = groups 2.5n..2.5n+2.5,
          # covered by pieces 0..n (piece p covers groups 3p..3p+3)
          xt = []
          y0 = y_p.tile([128, CO], F32, tag="y_sb", name="y_sb")
          done_chunks = 0
          stored_cols = 0
          for p in range(N_PC):
              w_piece_pipeline(p)
              if p == 0:
                  xt.extend(x_transpose(t) for t in range(2))
              cov = min(CO, (p + 1) * GPP * 128)     # wm columns ready
              if p == N_PC - 1:
                  cov = CO
              while done_chunks < N_CC and (done_chunks + 1) * CO_CHUNK <= cov:
                  y_chunk(0, done_chunks, y0, T0_ACT)
                  done_chunks += 1
              if done_chunks >= 2 and stored_cols == 0:
                  stored_cols = done_chunks * CO_CHUNK
                  nc.sync.dma_start(y_flat[0:128, :stored_cols],
                                    y0[:, :stored_cols])
          nc.sync.dma_start(y_flat[0:128, stored_cols:], y0[:, stored_cols:])

          # remaining x transposes
          xt += [x_transpose(t) for t in range(2, N_BT)]

          # bias_rep via rank-1 matmuls (only DVE chunks need it; first use
          # is t=1 -- build it behind t=0's epilogue)
          if T0_ACT and Y_DVE_OF_5 > 0:
              for n in range(N_CC):
                  sl = slice(n * CO_CHUNK, (n + 1) * CO_CHUNK)
                  ps = ps_y.tile([128, CO_CHUNK], F32, tag="ps", name="ps")
                  nc.tensor.matmul(ps[:], ones_r[:], bias_r[:, sl],
                                   start=True, stop=True)
                  nc.scalar.copy(bias_rep[:, sl], ps[:])

          for t in range(1, N_BT):
              y_sb = y_p.tile([128, CO], F32, tag="y_sb", name="y_sb")
              for n in range(N_CC):
                  on_act = (t * N_CC + n) % 5 >= Y_DVE_OF_5
                  y_chunk(t, n, y_sb, on_act)
              nc.sync.dma_start(y_flat[t * 128:(t + 1) * 128, :], y_sb[:])

      if rep_cm is not None:
          rep_cm.__exit__(None, None, None)

    _split_multi_waits(nc)
    return nc


def _get_nc(mode, repeat=1):
    key = (mode, repeat)
    if key not in _CACHE:
        _CACHE[key] = _build(mode, repeat)
    return _CACHE[key]


def kernel(x: np.ndarray, weight: np.ndarray, bias: np.ndarray,
           _trace: bool = False, _repeat: int = 1):
    nc = _get_nc(MM_MODE, _repeat)
    x = np.ascontiguousarray(x, dtype=np.float32)
    weight = np.ascontiguousarray(weight, dtype=np.float32)
    bias = np.ascontiguousarray(bias, dtype=np.float32)
    in_maps = [
        {"x": x[i * BS:(i + 1) * BS], "weight": weight, "bias": bias}
        for i in range(N_CORES)
    ]
    res = run_bass_kernel_spmd(nc, in_maps, list(range(N_CORES)), trace=_trace)
    out = np.concatenate([res.results[i]["y"] for i in range(N_CORES)], axis=0)
    if _trace:
        return out, res
    return out



# revision 3
# speedup vs baseline: 2.4422x; 2.4422x over previous
"""EntropyLinear Trainium2 kernel (8-core SPMD, batch-sharded), v2.

Computes, for x[B,IN], weight[C,OUT,IN], bias[C,1,OUT]:
    gamma[c,i]      = sum_o |W[c,o,i]|
    alpha_norm[c,i] = exp((gamma[c,i] - max_i gamma[c,i]) / T)
    y[b,c,o]        = sum_i x[b,i] * alpha_norm[c,i] * W[c,o,i] + bias[c,0,o]

Strategy vs v1: all-bf16 data path.  x and W are cast to bf16 on the host
and loaded PRE-TRANSPOSED via the hardware DMA xbar-transpose, so the PE
never transposes operands and nothing is evacuated from PSUM for them.
gamma comes straight off the transposed W with a single abs-fused DVE
reduction per half (no |W| materialization, no mask matmuls).  y is stored
as bf16 (half the store traffic) and upcast to f32 on the host.  Bias is
injected into PSUM by a rank-1 (ones x bias) matmul for the ACT-evacuated
chunks and fused into the DVE add for the DVE-evacuated chunks.

Per-core per-iteration budget (cost model): DMA ~14us, PE ~14us,
DVE ~13us, ACT ~11us.
"""

import os
from contextlib import ExitStack

import numpy as np
import ml_dtypes

import concourse.bass as bass
import concourse.tile as tile
from concourse import masks, mybir
from concourse.bass_utils import run_bass_kernel_spmd

# ---------------------------------------------------------------------------
# Workaround for this walrus build's 1-sync-wait-per-instruction limit:
# Tile's scheduler (and its exit drain) may attach several semaphore waits to
# one instruction; walrus codegen rejects that ("Too many sync wait
# commands").  Post-pass: hoist extra waits onto same-engine NOPs emitted
# immediately before the instruction (same engine stream => same semantics).
# ---------------------------------------------------------------------------


def _split_multi_waits(nc):
    for fn in nc.m.functions:
        for blk in fn.blocks:
            out = []
            changed = False
            for inst in list(blk.instructions):
                si = inst.sync_info
                waits = list(si.on_wait) if si is not None and si.on_wait else []
                if len(waits) > 1:
                    changed = True
                    for w in waits[:-1]:
                        nop = mybir.InstNoOp(
                            name=nc.get_next_instruction_name(), ins=[], outs=[])
                        nop.engine = inst.engine
                        nop.sync_info = mybir.SyncInfo(on_wait=[w], on_update=[])
                        nc.register_instruction(nop)
                        out.append(nop)
                    upd = list(si.on_update) if si.on_update else []
                    inst.sync_info = mybir.SyncInfo(
                        on_wait=[waits[-1]], on_update=upd)
                out.append(inst)
            if changed:
                blk.instructions = out

# ---------------------------------------------------------------------------

B, IN, OUT, C = 8192, 256, 32, 50
TEMPERATURE = 0.6
N_CORES = 8
BS = B // N_CORES          # rows of x per core
CO = C * OUT               # 1600 fused (class, out) columns
F32 = mybir.dt.float32
BF16 = mybir.dt.bfloat16

N_BT = BS // 128           # b-tiles per core (8)
N_KH = IN // 128           # contraction halves (2)
CO_CHUNK = 400             # psum chunk (1600B -> one psum bank)
N_CC = CO // CO_CHUNK      # chunks per b-tile (4)
N_ACT = int(os.environ.get("EL_N_ACT", "2"))  # chunks/tile evacuated on ACT

STAGGER = os.environ.get("EL_STAGGER", "0") == "1"
HINTS = os.environ.get("EL_HINTS", "0") == "1"

_CACHE = {}


def _build(repeat=1):
    nc = bass.Bass(trn_type="TRN2", target_bir_lowering=False, debug=False,
                   num_devices=N_CORES)
    x_d = nc.dram_tensor("x", [BS, IN], BF16, kind="ExternalInput").ap()
    w_d = nc.dram_tensor("weight", [C, OUT, IN], BF16, kind="ExternalInput").ap()
    b_d = nc.dram_tensor("bias", [1, CO], BF16, kind="ExternalInput").ap()
    y_d = nc.dram_tensor("y", [BS, C, OUT], BF16, kind="ExternalOutput").ap()

    w_flat = w_d.rearrange("c o i -> (c o) i")      # [1600, 256]
    y_flat = y_d.rearrange("b c o -> b (c o)")      # [BS, 1600]

    with tile.TileContext(nc) as tc, ExitStack() as ctx:
      const_p = ctx.enter_context(tc.tile_pool(name="const", bufs=1))
      w_p = ctx.enter_context(tc.tile_pool(name="w", bufs=2))
      x_p = ctx.enter_context(tc.tile_pool(name="x", bufs=2))
      small_p = ctx.enter_context(tc.tile_pool(name="small", bufs=2))
      y_p = ctx.enter_context(tc.tile_pool(name="y", bufs=3))
      ps_tp = ctx.enter_context(tc.tile_pool(name="ps_tp", bufs=1, space="PSUM"))
      ps_y = ctx.enter_context(tc.tile_pool(name="ps_y", bufs=6, space="PSUM"))

      hint_engines = ()
      if HINTS:
          hint_engines = (mybir.EngineType.PE, mybir.EngineType.DVE,
                          mybir.EngineType.Activation, mybir.EngineType.SP,
                          mybir.EngineType.Pool)
      rep_cm = (tc.For_i(0, repeat, 1, staggered_reset=STAGGER,
                         hint_engines=hint_engines)
                if repeat > 1 else None)
      if rep_cm is not None:
          rep_cm.__enter__()
      if True:
          # ---- constants ----
          ident = const_p.tile([128, 128], F32, tag="ident")
          masks.make_identity(nc, ident[:])
          ones_r = const_p.tile([1, 128], BF16, tag="ones_r")
          nc.vector.memset(ones_r[:], 1.0)

          # ---- loads (ACT HWDGE ring; stores go on the SP ring) ----
          wt = [w_p.tile([128, CO], BF16, tag=f"wt{h}", name=f"wt{h}")
                for h in range(N_KH)]
          for h in range(N_KH):
              nc.scalar.dma_start(wt[h][:], w_flat[:, h * 128:(h + 1) * 128],
                                  transpose=True)
          bias_r = const_p.tile([1, CO], BF16, tag="bias_r")
          nc.scalar.dma_start(bias_r[:], b_d)
          xt = [x_p.tile([128, BS], BF16, tag=f"xt{h}", name=f"xt{h}")
                for h in range(N_KH)]
          for h in range(N_KH):
              nc.scalar.dma_start(xt[h][:], x_d[:, h * 128:(h + 1) * 128],
                                  transpose=True)

          # ---- prologue: gamma -> alpha -> wm ----
          # gamma halves, i-major: gt[h][i, c] = sum_o |wt[h][i, (c,o)]|
          gt = [small_p.tile([128, C], F32, tag=f"gt{h}", name=f"gt{h}")
                for h in range(N_KH)]
          for h in range(N_KH):
              nc.vector.tensor_reduce(
                  gt[h][:], wt[h][:].rearrange("p (c o) -> p c o", o=OUT),
                  axis=mybir.AxisListType.X, op=mybir.AluOpType.add,
                  apply_absolute_value=True)
          # transpose to rows [c, i] for the per-class max
          grows = ps_tp.tile([C, IN], F32, tag="grows", name="grows")
          for h in range(N_KH):
              nc.tensor.transpose(grows[:, h * 128:(h + 1) * 128],
                                  gt[h][:], ident[:])
          gm = small_p.tile([C, 1], F32, tag="gm", name="gm")
          nc.vector.tensor_reduce(gm[:], grows[:],
                                  axis=mybir.AxisListType.X,
                                  op=mybir.AluOpType.max)
          nb = small_p.tile([C, 1], F32, tag="nb", name="nb")
          nc.vector.tensor_scalar_mul(nb[:], gm[:], -1.0 / TEMPERATURE)
          anp = small_p.tile([C, IN], F32, tag="anp", name="anp")
          nc.scalar.activation(anp[:], grows[:],
                               mybir.ActivationFunctionType.Exp,
                               bias=nb[:], scale=1.0 / TEMPERATURE)
          # back to i-major: ant[h][i, c], bf16
          antp = ps_tp.tile([128, 2 * C], F32, tag="antp", name="antp")
          for h in range(N_KH):
              nc.tensor.transpose(antp[:, h * C:(h + 1) * C],
                                  anp[:, h * 128:(h + 1) * 128],
                                  ident[:C, :C])
          ant = [small_p.tile([128, C], BF16, tag=f"ant{h}", name=f"ant{h}")
                 for h in range(N_KH)]
          for h in range(N_KH):
              nc.vector.tensor_copy(ant[h][:], antp[:, h * C:(h + 1) * C])
          # wm[h][i, (c,o)] = wt[h][i, (c,o)] * ant[h][i, c]
          wm = [w_p.tile([128, CO], BF16, tag=f"wm{h}", name=f"wm{h}")
                for h in range(N_KH)]
          for h in range(N_KH):
              nc.vector.tensor_tensor(
                  wm[h][:].rearrange("p (c o) -> p c o", o=OUT),
                  wt[h][:].rearrange("p (c o) -> p c o", o=OUT),
                  ant[h][:].unsqueeze(2).broadcast_to([128, C, OUT]),
                  op=mybir.AluOpType.mult)

          # bias_rep (f32) for the DVE-evacuated chunk columns
          ndve = N_CC - N_ACT
          bias_rep = const_p.tile([128, max(1, ndve * CO_CHUNK)], F32,
                                  tag="bias_rep")
          for n in range(N_ACT, N_CC):
              sl = slice(n * CO_CHUNK, (n + 1) * CO_CHUNK)
              dsl = slice((n - N_ACT) * CO_CHUNK, (n - N_ACT + 1) * CO_CHUNK)
              ps = ps_y.tile([128, CO_CHUNK], F32, tag="ps", name="ps")
              nc.tensor.matmul(ps[:], ones_r[:], bias_r[:, sl],
                               start=True, stop=True)
              nc.scalar.copy(bias_rep[:, dsl], ps[:])

          # ---- main loop over b-tiles ----
          for t in range(N_BT):
              y_sb = y_p.tile([128, CO], BF16, tag="y_sb", name="y_sb")
              pss = []
              for n in range(N_CC):
                  sl = slice(n * CO_CHUNK, (n + 1) * CO_CHUNK)
                  ps = ps_y.tile([128, CO_CHUNK], F32, tag="ps", name="ps")
                  pss.append(ps)
                  if n < N_ACT:
                      nc.tensor.matmul(ps[:], ones_r[:], bias_r[:, sl],
                                       start=True, stop=False)
              for h in range(N_KH):
                  xsl = xt[h][:, t * 128:(t + 1) * 128]
                  for n in range(N_CC):
                      sl = slice(n * CO_CHUNK, (n + 1) * CO_CHUNK)
                      nc.tensor.matmul(pss[n][:], xsl, wm[h][:, sl],
                                       start=(h == 0 and n >= N_ACT),
                                       stop=(h == N_KH - 1))
              for n in range(N_CC):
                  sl = slice(n * CO_CHUNK, (n + 1) * CO_CHUNK)
                  if n < N_ACT:
                      nc.scalar.copy(y_sb[:, sl], pss[n][:])
                  else:
                      dsl = slice((n - N_ACT) * CO_CHUNK,
                                  (n - N_ACT + 1) * CO_CHUNK)
                      nc.vector.tensor_tensor(y_sb[:, sl], pss[n][:],
                                              bias_rep[:, dsl],
                                              op=mybir.AluOpType.add)
              nc.sync.dma_start(y_flat[t * 128:(t + 1) * 128, :], y_sb[:])

      if rep_cm is not None:
          rep_cm.__exit__(None, None, None)

    _split_multi_waits(nc)
    return nc


def _get_nc(repeat=1):
    if repeat not in _CACHE:
        _CACHE[repeat] = _build(repeat)
    return _CACHE[repeat]


def kernel(x: np.ndarray, weight: np.ndarray, bias: np.ndarray,
           _trace: bool = False, _repeat: int = 1):
    nc = _get_nc(_repeat)
    xb = np.ascontiguousarray(x, dtype=np.float32).astype(ml_dtypes.bfloat16)
    wb = np.ascontiguousarray(weight, dtype=np.float32).astype(ml_dtypes.bfloat16)
    bb = np.ascontiguousarray(
        bias, dtype=np.float32).reshape(1, CO).astype(ml_dtypes.bfloat16)
    in_maps = [
        {"x": xb[i * BS:(i + 1) * BS], "weight": wb, "bias": bb}
        for i in range(N_CORES)
    ]
    res = run_bass_kernel_spmd(nc, in_maps, list(range(N_CORES)), trace=_trace)
    out = np.concatenate(
        [np.asarray(res.results[i]["y"]) for i in range(N_CORES)],
        axis=0).astype(np.float32)
    if _trace:
        return out, res
    return out


# revision 18
# speedup vs baseline: 3.1654x; 1.2961x over previous
"""EntropyLinear Trainium2 kernel (8-core SPMD, batch-sharded), v3.

Computes, for x[B,IN], weight[C,OUT,IN], bias[C,1,OUT]:
    gamma[c,i]      = sum_o |W[c,o,i]|
    alpha_norm[c,i] = exp((gamma[c,i] - max_i gamma[c,i]) / T)
    y[b,c,o]        = sum_i x[b,i] * alpha_norm[c,i] * W[c,o,i] + bias[c,0,o]

All-bf16 data path: x and W are cast to bf16 on the host and loaded
PRE-TRANSPOSED via the hardware DMA xbar-transpose (no PE operand
transposes, no PSUM evacuation for operands).  gamma comes straight off
the transposed W with abs-fused DVE reductions (no |W| pass, no mask
matmuls).  y is stored bf16 (half the store traffic) and upcast on the
host.  Bias is injected by rank-1 (ones x bias) matmuls for the
ACT-evacuated psum chunks and fused into the DVE add for the rest.

Loads/operands are split (W by co-half, x by b-half, wm per chunk) so
downstream work starts as soon as its slice lands.  The repeat loop used
for timing supports staggered semaphore reset + branch hints with stage
boundaries placed between b-tile groups, letting iteration i+1's loads
and prologue overlap iteration i's tail tiles.
"""

import os
from contextlib import ExitStack

import numpy as np
import ml_dtypes

import concourse.bass as bass
import concourse.tile as tile
from concourse import masks, mybir
from concourse.bass_utils import run_bass_kernel_spmd

# ---------------------------------------------------------------------------
# Workaround for this walrus build's 1-sync-wait-per-instruction limit:
# Tile's scheduler (and its exit drain) may attach several semaphore waits to
# one instruction; walrus codegen rejects that ("Too many sync wait
# commands").  Post-pass: hoist extra waits onto same-engine NOPs emitted
# immediately before the instruction (same engine stream => same semantics).
# ---------------------------------------------------------------------------


def _split_multi_waits(nc):
    for fn in nc.m.functions:
        for blk in fn.blocks:
            out = []
            changed = False
            for inst in list(blk.instructions):
                si = inst.sync_info
                waits = list(si.on_wait) if si is not None and si.on_wait else []
                if len(waits) > 1:
                    changed = True
                    for w in waits[:-1]:
                        nop = mybir.InstNoOp(
                            name=nc.get_next_instruction_name(), ins=[], outs=[])
                        nop.engine = inst.engine
                        nop.sync_info = mybir.SyncInfo(on_wait=[w], on_update=[])
                        nc.register_instruction(nop)
                        out.append(nop)
                    upd = list(si.on_update) if si.on_update else []
                    inst.sync_info = mybir.SyncInfo(
                        on_wait=[waits[-1]], on_update=upd)
                out.append(inst)
            if changed:
                blk.instructions = out

# ---------------------------------------------------------------------------

B, IN, OUT, C = 8192, 256, 32, 50
TEMPERATURE = 0.6
N_CORES = 8
BS = B // N_CORES          # rows of x per core
CO = C * OUT               # 1600 fused (class, out) columns
F32 = mybir.dt.float32
BF16 = mybir.dt.bfloat16

N_BT = BS // 128           # b-tiles per core (8)
N_KH = IN // 128           # contraction halves (2)
# W is loaded in two co-pieces split at class 32 (so the gamma transposes
# land at psum base partitions 0/32); psum chunks nest inside the pieces.
WCLS = (32, 18)                # classes per W co-piece
WC0 = (0, 32)                  # class base of each piece
WCOL0 = (0, 32 * OUT)          # column base of each piece (0, 1024)
CHUNK_NCLS = (16, 16, 9, 9)    # classes per psum chunk
CHUNK_C0 = (0, 16, 32, 41)
CHUNK_J = (0, 0, 1, 1)         # which W co-piece holds each chunk
CHUNK_COL0 = tuple(c * OUT for c in CHUNK_C0)          # (0, 512, 1024, 1312)
CHUNK_COLS = tuple(n * OUT for n in CHUNK_NCLS)        # (512, 512, 288, 288)
N_CC = len(CHUNK_NCLS)     # chunks per b-tile (4)
# chunks evacuated on ACT (bias via rank-1) vs DVE (bias fused in the add)
ACT_CHUNKS = (0, 2)
DVE_CHUNKS = (1, 3)

STAGGER = os.environ.get("EL_STAGGER", "0") == "1"
HINTS = os.environ.get("EL_HINTS", "1") == "1"
UNROLL = int(os.environ.get("EL_UNROLL", "4"))
# b-tile indices before which a staggered-reset stage boundary is placed
STAGES = tuple(int(s) for s in os.environ.get("EL_STAGES", "0,2,4").split(","))

_CACHE = {}


def _build(repeat=1):
    nc = bass.Bass(trn_type="TRN2", target_bir_lowering=False, debug=False,
                   num_devices=N_CORES)
    x_d = nc.dram_tensor("x", [BS, IN], BF16, kind="ExternalInput").ap()
    w_d = nc.dram_tensor("weight", [C, OUT, IN], BF16, kind="ExternalInput").ap()
    b_d = nc.dram_tensor("bias", [1, CO], BF16, kind="ExternalInput").ap()
    y_d = nc.dram_tensor("y", [BS, C, OUT], BF16, kind="ExternalOutput").ap()

    w_flat = w_d.rearrange("c o i -> (c o) i")      # [1600, 256]
    y_flat = y_d.rearrange("b c o -> b (c o)")      # [BS, 1600]

    BH = BS // 2            # b rows per x load piece (512)

    with tile.TileContext(nc) as tc, ExitStack() as ctx:
      const_p = ctx.enter_context(tc.tile_pool(name="const", bufs=2))
      w_p = ctx.enter_context(tc.tile_pool(name="w", bufs=2))
      x_p = ctx.enter_context(tc.tile_pool(name="x", bufs=2))
      small_p = ctx.enter_context(tc.tile_pool(name="small", bufs=2))
      y_p = ctx.enter_context(tc.tile_pool(name="y", bufs=4))
      ps_tp = ctx.enter_context(tc.tile_pool(name="ps_tp", bufs=1, space="PSUM"))
      ps_y = ctx.enter_context(tc.tile_pool(name="ps_y", bufs=6, space="PSUM"))

      hint_engines = ()
      if HINTS:
          hint_engines = (mybir.EngineType.PE, mybir.EngineType.DVE,
                          mybir.EngineType.Activation, mybir.EngineType.SP,
                          mybir.EngineType.Pool)
      unroll = UNROLL if repeat > 1 and repeat % UNROLL == 0 else 1
      rep_cm = (tc.For_i(0, repeat // unroll, 1, staggered_reset=STAGGER,
                         hint_engines=hint_engines)
                if repeat > 1 else None)
      if rep_cm is not None:
          rep_cm.__enter__()
      for _u in range(unroll if repeat > 1 else 1):
          # ---- constants ----
          ident = const_p.tile([128, 128], F32, tag="ident", name="ident")
          masks.make_identity(nc, ident[:])
          ones_r = const_p.tile([1, 128], BF16, tag="ones_r", name="ones_r")
          nc.vector.memset(ones_r[:], 1.0)

          # ---- loads (ACT HWDGE ring; stores go on the SP ring) ----
          # W transposed, split in co-pieces per i-half so gamma/wm start early
          wt = [[w_p.tile([128, WCLS[j] * OUT], BF16, tag=f"wt{h}{j}",
                          name=f"wt{h}{j}")
                 for j in range(2)] for h in range(N_KH)]
          for h in range(N_KH):
              for j in range(2):
                  nc.scalar.dma_start(
                      wt[h][j][:],
                      w_flat[WCOL0[j]:WCOL0[j] + WCLS[j] * OUT,
                             h * 128:(h + 1) * 128],
                      transpose=True)
          bias_r = const_p.tile([1, CO], BF16, tag="bias_r", name="bias_r")
          nc.scalar.dma_start(bias_r[:], b_d)
          # x transposed, split in b-halves
          xt = [[x_p.tile([128, BH], BF16, tag=f"xt{h}{j}", name=f"xt{h}{j}")
                 for j in range(2)] for h in range(N_KH)]
          for h in range(N_KH):
              for j in range(2):
                  nc.scalar.dma_start(
                      xt[h][j][:],
                      x_d[j * BH:(j + 1) * BH, h * 128:(h + 1) * 128],
                      transpose=True)

          def xsl(h, t):
              return xt[h][t // 4][:, (t % 4) * 128:(t % 4 + 1) * 128]

          # ---- prologue: gamma -> alpha -> wm ----
          # gamma quarters, i-major: gt[h][j][i, c'] = sum_o |wt[h][j]|
          gt = [[small_p.tile([128, WCLS[j]], F32, tag=f"gt{h}{j}",
                              name=f"gt{h}{j}")
                 for j in range(2)] for h in range(N_KH)]
          for h in range(N_KH):
              for j in range(2):
                  nc.vector.tensor_reduce(
                      gt[h][j][:],
                      wt[h][j][:].rearrange("p (c o) -> p c o", o=OUT),
                      axis=mybir.AxisListType.X, op=mybir.AluOpType.add,
                      apply_absolute_value=True)
          # transpose to rows [c', i] for the per-class max; transpose-mode
          # matmuls must write psum base partition 0, so each co-piece gets
          # its own row range packed into one [32, 512] psum bank
          grows2 = ps_tp.tile([32, 2 * IN], F32, tag="grows2", name="grows2")
          grows = [grows2[:WCLS[0], :IN], grows2[:WCLS[1], IN:]]
          for h in range(N_KH):
              for j in range(2):
                  nc.tensor.transpose(
                      grows[j][:, h * 128:(h + 1) * 128],
                      gt[h][j][:], ident[:])
          anp = [small_p.tile([WCLS[j], IN], F32, tag=f"anp{j}",
                              name=f"anp{j}") for j in range(2)]
          for j in range(2):
              gm = small_p.tile([WCLS[j], 1], F32, tag=f"gm{j}",
                                name=f"gm{j}")
              nc.vector.tensor_reduce(gm[:], grows[j][:],
                                      axis=mybir.AxisListType.X,
                                      op=mybir.AluOpType.max)
              nb = small_p.tile([WCLS[j], 1], F32, tag=f"nb{j}",
                                name=f"nb{j}")
              nc.vector.tensor_scalar_mul(nb[:], gm[:], -1.0 / TEMPERATURE)
              nc.scalar.activation(anp[j][:], grows[j][:],
                                   mybir.ActivationFunctionType.Exp,
                                   bias=nb[:], scale=1.0 / TEMPERATURE)
          # back to i-major: ant[h][i, c], bf16
          antp = ps_tp.tile([128, 2 * C], F32, tag="antp", name="antp")
          for h in range(N_KH):
              for j in range(2):
                  nc.tensor.transpose(
                      antp[:, h * C + WC0[j]:h * C + WC0[j] + WCLS[j]],
                      anp[j][:, h * 128:(h + 1) * 128],
                      ident[:WCLS[j], :WCLS[j]])
          ant = [small_p.tile([128, C], BF16, tag=f"ant{h}", name=f"ant{h}")
                 for h in range(N_KH)]
          for h in range(N_KH):
              nc.vector.tensor_copy(ant[h][:], antp[:, h * C:(h + 1) * C])

          # wm per (h, chunk): wm[h][n][i, co'] = wt * ant (bcast over o)
          wm = [[w_p.tile([128, CHUNK_COLS[n]], BF16, tag=f"wm{h}{n}",
                          name=f"wm{h}{n}")
                 for n in range(N_CC)] for h in range(N_KH)]
          for h in range(N_KH):
              for n in range(N_CC):
                  j = CHUNK_J[n]                     # wt co-piece of chunk n
                  base = CHUNK_COL0[n] - WCOL0[j]    # col offset in the piece
                  c0, ncls = CHUNK_C0[n], CHUNK_NCLS[n]
                  nc.vector.tensor_tensor(
                      wm[h][n][:].rearrange("p (c o) -> p c o", o=OUT),
                      wt[h][j][:, base:base + CHUNK_COLS[n]].rearrange(
                          "p (c o) -> p c o", o=OUT),
                      ant[h][:, c0:c0 + ncls].unsqueeze(2).broadcast_to(
                          [128, ncls, OUT]),
                      op=mybir.AluOpType.mult)

          # bias_rep (f32) for the DVE-evacuated chunk columns
          nrep = sum(CHUNK_COLS[n] for n in DVE_CHUNKS)
          bias_rep = const_p.tile([128, max(1, nrep)],
                                  F32, tag="bias_rep", name="bias_rep")
          dcol = 0
          for n in DVE_CHUNKS:
              sl = slice(CHUNK_COL0[n], CHUNK_COL0[n] + CHUNK_COLS[n])
              dsl = slice(dcol, dcol + CHUNK_COLS[n])
              dcol += CHUNK_COLS[n]
              ps = ps_y.tile([128, 512], F32, tag="ps", name="ps")
              nc.tensor.matmul(ps[:, :CHUNK_COLS[n]], ones_r[:], bias_r[:, sl],
                               start=True, stop=True)
              nc.scalar.copy(bias_rep[:, dsl], ps[:, :CHUNK_COLS[n]])

          # ---- main loop over b-tiles ----
          for t in range(N_BT):
              if rep_cm is not None and STAGGER and t in STAGES:
                  tc.stage_boundary()
              y_sb = y_p.tile([128, CO], BF16, tag="y_sb", name="y_sb")
              pss = []
              for n in range(N_CC):
                  sl = slice(CHUNK_COL0[n], CHUNK_COL0[n] + CHUNK_COLS[n])
                  ps = ps_y.tile([128, 512], F32, tag="ps", name="ps")
                  pss.append(ps[:, :CHUNK_COLS[n]])
                  if n in ACT_CHUNKS:
                      nc.tensor.matmul(pss[n], ones_r[:], bias_r[:, sl],
                                       start=True, stop=False)
              for h in range(N_KH):
                  for n in range(N_CC):
                      nc.tensor.matmul(pss[n], xsl(h, t), wm[h][n][:],
                                       start=(h == 0 and n in DVE_CHUNKS),
                                       stop=(h == N_KH - 1))
              dcol = 0
              for n in range(N_CC):
                  sl = slice(CHUNK_COL0[n], CHUNK_COL0[n] + CHUNK_COLS[n])
                  if n in ACT_CHUNKS:
                      nc.scalar.copy(y_sb[:, sl], pss[n])
                  else:
                      dsl = slice(dcol, dcol + CHUNK_COLS[n])
                      dcol += CHUNK_COLS[n]
                      nc.vector.tensor_tensor(y_sb[:, sl], pss[n],
                                              bias_rep[:, dsl],
                                              op=mybir.AluOpType.add)
              nc.sync.dma_start(y_flat[t * 128:(t + 1) * 128, :], y_sb[:])

      if rep_cm is not None:
          rep_cm.__exit__(None, None, None)

    _split_multi_waits(nc)
    return nc


def _get_nc(repeat=1):
    if repeat not in _CACHE:
        _CACHE[repeat] = _build(repeat)
    return _CACHE[repeat]


def kernel(x: np.ndarray, weight: np.ndarray, bias: np.ndarray,
           _trace: bool = False, _repeat: int = 1):
    nc = _get_nc(_repeat)
    xb = np.ascontiguousarray(x, dtype=np.float32).astype(ml_dtypes.bfloat16)
    wb = np.ascontiguousarray(weight, dtype=np.float32).astype(ml_dtypes.bfloat16)
    bb = np.ascontiguousarray(
        bias, dtype=np.float32).reshape(1, CO).astype(ml_dtypes.bfloat16)
    in_maps = [
        {"x": xb[i * BS:(i + 1) * BS], "weight": wb, "bias": bb}
        for i in range(N_CORES)
    ]
    res = run_bass_kernel_spmd(nc, in_maps, list(range(N_CORES)), trace=_trace)
    out = np.concatenate(
        [np.asarray(res.results[i]["y"]) for i in range(N_CORES)],
        axis=0).astype(np.float32)
    if _trace:
        return out, res
    return out
